# revision 14
# baseline (speedup 1.0000x reference)
"""AGREE group-recommendation kernel for 8 TRN2 NeuronCores.

Data-parallel: 8192 groups sharded 1024/core. Per core:
  - indirect-DMA gather of member embeddings (bf16, b-layout:
    partition = group-within-tile, 8 tiles x 128 groups x 50 members)
  - attention MLP via PE transposes + block-diag matmuls
  - masked softmax (no max-subtraction: logits are tiny by construction)
  - weighted member sum on DVE, prediction MLP on PE
Host side only reshapes/casts inputs and concatenates outputs.
"""

import contextlib
import os

import numpy as np
import ml_dtypes

from concourse import bass, mybir
from concourse.bass import IndirectOffsetOnAxis
from concourse.bass_utils import run_bass_kernel_spmd

F32 = mybir.dt.float32
BF16 = mybir.dt.bfloat16
I32 = mybir.dt.int32

NUM_USERS = 100000
NUM_ITEMS = 50000
EMB = 64
B = 8192
MAXM = 50
ATT_H = 16
PRED_H = 8
NCORES = 8
BL = B // NCORES          # 1024 groups per core
T = 8                     # tiles per core
TG = 128                  # groups per tile
HALVES = ((0, 13), (13, 25))  # 2-member-chunk ranges per half-tile
NEG = -30000.0            # additive mask for invalid members
RELU = mybir.ActivationFunctionType.Relu
EXP = mybir.ActivationFunctionType.Exp
TANH = mybir.ActivationFunctionType.Tanh
MULT = mybir.AluOpType.mult
ADD = mybir.AluOpType.add

# --- precomputed semaphore schedules (must match emission order below) ---
PE_ORDER = ["itemT", "T0", "z0", "lg0", "T1", "z1", "lg1", "nT", "ph", "phT", "y"]
DV_ORDER = ["itemTe", "memT0", "memT1", "lm", "new", "nTe", "phTe", "yd"]
AC_ORDER = ["zr0", "zr1", "exp", "phr", "y"]


def _marks(order):
    m = {}
    v = 0
    for t in range(T):
        for k in order:
            v += 1
            m[(k, t)] = v
    return m


PE_M = _marks(PE_ORDER)
DV_M = _marks(DV_ORDER)
AC_M = _marks(AC_ORDER)


def build_nc(pred_b2: float):
    nc = bass.Bass()

    ut = nc.declare_dram_parameter("ut", [NUM_USERS, EMB], BF16, False)
    it = nc.declare_dram_parameter("it", [NUM_ITEMS, EMB], BF16, False)
    midx = nc.declare_dram_parameter("midx", [128, T * MAXM], I32, False)
    iidx = nc.declare_dram_parameter("iidx", [128, T], I32, False)
    madd = nc.declare_dram_parameter("madd", [128, T, MAXM], F32, False)
    w1u2 = nc.declare_dram_parameter("w1u2", [128, 2 * ATT_H], BF16, False)
    w1i2 = nc.declare_dram_parameter("w1i2", [EMB, 2 * ATT_H], BF16, False)
    b1c = nc.declare_dram_parameter("b1c", [2 * ATT_H, 1], F32, False)
    w2b = nc.declare_dram_parameter("w2b", [2 * ATT_H, 2], BF16, False)
    pw1a = nc.declare_dram_parameter("pw1a", [128, PRED_H], BF16, False)
    pw1b = nc.declare_dram_parameter("pw1b", [EMB, PRED_H], BF16, False)
    pb1r = nc.declare_dram_parameter("pb1r", [1, PRED_H], BF16, False)
    ones1 = nc.declare_dram_parameter("ones1", [1, 128], BF16, False)
    pw2 = nc.declare_dram_parameter("pw2", [PRED_H, 1], BF16, False)
    ident = nc.declare_dram_parameter("ident", [128, 128], BF16, False)
    out = nc.declare_dram_parameter("out", [128, T], F32, True)
    DBG = bool(int(os.environ.get("KERNEL_DEBUG", "0")))
    if DBG:
        d_emb = nc.declare_dram_parameter("d_emb", [128, MAXM, EMB], BF16, True)
        d_memT = nc.declare_dram_parameter("d_memT", [128, 12 * 128], BF16, True)
        d_zr = nc.declare_dram_parameter("d_zr", [2 * ATT_H, 12 * 128], BF16, True)
        d_itemT = nc.declare_dram_parameter("d_itemT", [EMB, 128], BF16, True)
        d_lm = nc.declare_dram_parameter("d_lm", [128, MAXM], F32, True)
        d_e = nc.declare_dram_parameter("d_e", [128, MAXM], F32, True)
        d_graw = nc.declare_dram_parameter("d_graw", [128, EMB], F32, True)
        d_new = nc.declare_dram_parameter("d_new", [128, 3 * EMB], BF16, True)

    ctx = contextlib.ExitStack()
    sb = ctx.enter_context
    emb_sb = sb(nc.sbuf_tensor("emb_sb", [128, T * MAXM, EMB], BF16))
    item_sb = sb(nc.sbuf_tensor("item_sb", [128, T, EMB], BF16))
    midx_sb = sb(nc.sbuf_tensor("midx_sb", [128, T * MAXM], I32))
    iidx_sb = sb(nc.sbuf_tensor("iidx_sb", [128, T], I32))
    madd_sb = sb(nc.sbuf_tensor("madd_sb", [128, T, MAXM], F32))
    w1u2_sb = sb(nc.sbuf_tensor("w1u2_sb", [128, 2 * ATT_H], BF16))
    w1i2_sb = sb(nc.sbuf_tensor("w1i2_sb", [EMB, 2 * ATT_H], BF16))
    b1c_sb = sb(nc.sbuf_tensor("b1c_sb", [2 * ATT_H, 1], F32))
    w2b_sb = sb(nc.sbuf_tensor("w2b_sb", [2 * ATT_H, 2], BF16))
    pw1a_sb = sb(nc.sbuf_tensor("pw1a_sb", [128, PRED_H], BF16))
    pw1b_sb = sb(nc.sbuf_tensor("pw1b_sb", [EMB, PRED_H], BF16))
    pb1r_sb = sb(nc.sbuf_tensor("pb1r_sb", [1, PRED_H], BF16))
    ones1_sb = sb(nc.sbuf_tensor("ones1_sb", [1, 128], BF16))
    pw2_sb = sb(nc.sbuf_tensor("pw2_sb", [PRED_H, 1], BF16))
    ident_sb = sb(nc.sbuf_tensor("ident_sb", [128, 128], BF16))

    itemT_sb = sb(nc.sbuf_tensor("itemT_sb", [EMB, 128], BF16))
    memT_sb = [sb(nc.sbuf_tensor(f"memT{h}_sb", [128, 13 * 128], BF16))
               for h in range(2)]
    zr_sb = [sb(nc.sbuf_tensor(f"zr{h}_sb", [2 * ATT_H, 13 * 128], BF16))
             for h in range(2)]
    lm_sb = sb(nc.sbuf_tensor("lm_sb", [128, MAXM], F32))
    e_sb = sb(nc.sbuf_tensor("e_sb", [128, MAXM], F32))
    ssum_sb = sb(nc.sbuf_tensor("ssum_sb", [128, 1], F32))
    rre_sb = sb(nc.sbuf_tensor("rre_sb", [128, 1], F32))
    prod_sb = sb(nc.sbuf_tensor("prod_sb", [128, MAXM, EMB], F32))
    new_sb = sb(nc.sbuf_tensor("new_sb", [128, 3 * EMB], BF16))
    nT1_sb = sb(nc.sbuf_tensor("nT1_sb", [128, 128], BF16))
    nT2_sb = sb(nc.sbuf_tensor("nT2_sb", [EMB, 128], BF16))
    phs_sb = sb(nc.sbuf_tensor("phs_sb", [128, PRED_H], BF16))
    phT_sb = sb(nc.sbuf_tensor("phT_sb", [PRED_H, 128], BF16))
    graw_sb = sb(nc.sbuf_tensor("graw_sb", [128, EMB], F32))
    ytanh_sb = sb(nc.sbuf_tensor("ytanh_sb", [128, 1], F32))
    yall_sb = sb(nc.sbuf_tensor("yall_sb", [128, T], F32))

    ps_tr = sb(nc.psum_tensor("ps_tr", [128, 13 * 128], BF16))
    ps_z = sb(nc.psum_tensor("ps_z", [2 * ATT_H, 13 * 128], F32))
    ps_trs = sb(nc.psum_tensor("ps_trs", [128, 256], BF16))
    # ps_trs carve (bf16): cols 0:128 itemT/nT1, 128:256 nT2/phT
    ps_sm = sb(nc.psum_tensor("ps_sm", [128, 80], F32))
    # ps_sm carve (f32): cols 0:64 logits, 64:72 ph, 72:73 y

    s_c = ctx.enter_context(nc.semaphore("s_c"))
    s_ci = ctx.enter_context(nc.semaphore("s_ci"))
    s_dd = ctx.enter_context(nc.semaphore("s_dd"))
    s_git = ctx.enter_context(nc.semaphore("s_git"))
    s_gt = [ctx.enter_context(nc.semaphore(f"s_g{t}")) for t in range(T)]
    s_pe = ctx.enter_context(nc.semaphore("s_pe"))
    s_dv = ctx.enter_context(nc.semaphore("s_dv"))
    s_ac = ctx.enter_context(nc.semaphore("s_ac"))
    s_out = ctx.enter_context(nc.semaphore("s_out"))

    consts = [
        (madd_sb, madd), (w1u2_sb, w1u2),
        (w1i2_sb, w1i2), (b1c_sb, b1c), (w2b_sb, w2b), (pw1a_sb, pw1a),
        (pw1b_sb, pw1b), (pb1r_sb, pb1r), (ones1_sb, ones1), (pw2_sb, pw2),
        (ident_sb, ident),
    ]
    NC_ALL = 16 * len(consts)

    with nc.Block() as block:

        @block.sync
        def _(sync):
            sync.dma_start(out=midx_sb[:], in_=midx[:]).then_inc(s_ci, 16)
            sync.dma_start(out=iidx_sb[:], in_=iidx[:]).then_inc(s_ci, 16)
            for dst, src in consts:
                sync.dma_start(out=dst[:], in_=src[:]).then_inc(s_c, 16)

        @block.gpsimd
        def _(gp):
            gp.wait_ge(s_ci, 32)  # midx + iidx loaded
            for t in range(T):
                gp.indirect_dma_start(
                    out=item_sb[:, t, :], out_offset=None, in_=it[:],
                    in_offset=IndirectOffsetOnAxis(
                        ap=iidx_sb[:, t:t + 1], axis=0),
                ).then_inc(s_git, 16)
            # walrus only supports one gathered row per partition per
            # indirect DMA -> column-wise gathers (128 rows each)
            for t in range(T):
                for s in range(MAXM):
                    gp.indirect_dma_start(
                        out=emb_sb[:, t * MAXM + s, :], out_offset=None,
                        in_=ut[:],
                        in_offset=IndirectOffsetOnAxis(
                            ap=midx_sb[:, t * MAXM + s:t * MAXM + s + 1],
                            axis=0),
                    ).then_inc(s_gt[t], 16)

        @block.tensor
        def _(pe):
            pe.wait_ge(s_c, NC_ALL)
            for t in range(T):
                if t == 0:
                    pe.wait_ge(s_git, 16 * T)
                pe.wait_ge(s_gt[t], 16 * MAXM)
                if t > 0:
                    pe.wait_ge(s_dv, DV_M[("nTe", t - 1)])
                pe.matmul(out=ps_trs[0:EMB, 0:128], lhsT=item_sb[:, t, :],
                          rhs=ident_sb[:], is_transpose=True,
                          start=True, stop=True).then_inc(s_pe, 1)  # itemT
                for h, (cs, ce) in enumerate(HALVES):
                    nch = ce - cs
                    ncol = nch * 128
                    for c in range(cs, ce):
                        i = pe.matmul(
                            out=ps_tr[:, (c - cs) * 128:(c - cs + 1) * 128],
                            lhsT=emb_sb[:, t * MAXM + 2 * c:t * MAXM + 2 * c + 2, :],
                            rhs=ident_sb[:], is_transpose=True,
                            start=True, stop=True)
                    i.then_inc(s_pe, 1)                             # T{h}
                    pe.wait_ge(s_dv, DV_M[(f"memT{h}", t)])
                    if h == 0:
                        pe.wait_ge(s_dv, DV_M[("itemTe", t)])
                    # psum-bank (512 f32) chunked z matmuls
                    for lo in range(0, ncol, 512):
                        w = min(512, ncol - lo)
                        pe.matmul(out=ps_z[:, lo:lo + w], lhsT=w1u2_sb[:],
                                  rhs=memT_sb[h][:, lo:lo + w],
                                  start=True, stop=False)
                        for p in range(lo, lo + w, 128):
                            i = pe.matmul(
                                out=ps_z[:, p:p + 128],
                                lhsT=w1i2_sb[:], rhs=itemT_sb[:],
                                start=False, stop=(p + 128 >= lo + w))
                    i.then_inc(s_pe, 1)                             # z{h}
                    pe.wait_ge(s_ac, AC_M[(f"zr{h}", t)])
                    if h == 0 and t > 0:
                        pe.wait_ge(s_dv, DV_M[("lm", t - 1)])
                    for c in range(nch):
                        m0 = 2 * (cs + c)
                        i = pe.matmul(out=ps_sm[:, m0:m0 + 2],
                                      lhsT=zr_sb[h][:, c * 128:(c + 1) * 128],
                                      rhs=w2b_sb[:], start=True, stop=True)
                    i.then_inc(s_pe, 1)                             # lg{h}
                # prediction MLP
                pe.wait_ge(s_dv, DV_M[("new", t)])
                pe.matmul(out=ps_trs[:, 0:128], lhsT=new_sb[:, 0:128],
                          rhs=ident_sb[:], is_transpose=True,
                          start=True, stop=True)
                pe.matmul(out=ps_trs[0:EMB, 128:256], lhsT=new_sb[:, 128:192],
                          rhs=ident_sb[:], is_transpose=True,
                          start=True, stop=True).then_inc(s_pe, 1)  # nT
                pe.wait_ge(s_dv, DV_M[("nTe", t)])
                pe.matmul(out=ps_sm[:, 64:72], lhsT=nT1_sb[:],
                          rhs=pw1a_sb[:], start=True, stop=False)
                pe.matmul(out=ps_sm[:, 64:72], lhsT=nT2_sb[:],
                          rhs=pw1b_sb[:], start=False, stop=False)
                pe.matmul(out=ps_sm[:, 64:72], lhsT=ones1_sb[:],
                          rhs=pb1r_sb[:], start=False,
                          stop=True).then_inc(s_pe, 1)              # ph
                pe.wait_ge(s_ac, AC_M[("phr", t)])
                if t > 0:
                    pe.wait_ge(s_ac, AC_M[("y", t - 1)])
                pe.matmul(out=ps_trs[0:PRED_H, 128:256], lhsT=phs_sb[:],
                          rhs=ident_sb[:], is_transpose=True,
                          start=True, stop=True).then_inc(s_pe, 1)  # phT
                pe.wait_ge(s_dv, DV_M[("phTe", t)])
                pe.matmul(out=ps_sm[:, 72:73], lhsT=phT_sb[:],
                          rhs=pw2_sb[:], start=True,
                          stop=True).then_inc(s_pe, 1)              # y

        @block.vector
        def _(dv):
            dd = [0]
            dv.wait_ge(s_c, NC_ALL)
            for t in range(T):
                dv.wait_ge(s_pe, PE_M[("itemT", t)])
                dv.tensor_copy(itemT_sb[:],
                               ps_trs[0:EMB, 0:128]).then_inc(s_dv, 1)  # itemTe
                for h, (cs, ce) in enumerate(HALVES):
                    ncol = (ce - cs) * 128
                    dv.wait_ge(s_pe, PE_M[(f"T{h}", t)])
                    dv.tensor_copy(memT_sb[h][:, 0:ncol],
                                   ps_tr[:, 0:ncol]).then_inc(s_dv, 1)  # memT{h}
                dv.wait_ge(s_pe, PE_M[("lg1", t)])
                dv.tensor_add(lm_sb[:], ps_sm[:, 0:MAXM],
                              madd_sb[:, t, :]).then_inc(s_dv, 1)     # lm
                dv.wait_ge(s_ac, AC_M[("exp", t)])
                dv.reduce_sum(ssum_sb[:], e_sb[:],
                              axis=mybir.AxisListType.X).then_inc(s_dd, 1)
                dd[0] += 1
                dv.tensor_tensor(
                    out=prod_sb[:], in0=emb_sb[:, t * MAXM:(t + 1) * MAXM, :],
                    in1=e_sb[:].to_broadcast([128, MAXM, EMB]),
                    op=MULT).then_inc(s_dd, 1)
                dd[0] += 1
                dv.wait_ge(s_dd, dd[0] - 1)
                dv.reciprocal(rre_sb[:], ssum_sb[:]).then_inc(s_dd, 1)
                dd[0] += 1
                dv.wait_ge(s_dd, dd[0] - 1)
                dv.tensor_reduce(
                    out=graw_sb[:],
                    in_=prod_sb[:].rearrange("p m d -> p d m"),
                    axis=mybir.AxisListType.X,
                    op=ADD).then_inc(s_dd, 1)
                dd[0] += 1
                dv.wait_ge(s_dd, dd[0])
                dv.tensor_scalar(out=new_sb[:, EMB:2 * EMB],
                                 in0=graw_sb[:], scalar1=rre_sb[:],
                                 scalar2=None, op0=MULT).then_inc(s_dd, 1)
                dd[0] += 1
                dv.wait_ge(s_dd, dd[0])
                dv.tensor_tensor(out=new_sb[:, 0:EMB],
                                 in0=new_sb[:, EMB:2 * EMB],
                                 in1=item_sb[:, t, :], op=MULT)
                dv.tensor_copy(new_sb[:, 2 * EMB:3 * EMB],
                               item_sb[:, t, :]).then_inc(s_dv, 1)    # new
                dv.wait_ge(s_pe, PE_M[("nT", t)])
                dv.tensor_copy(nT1_sb[:], ps_trs[:, 0:128])
                dv.tensor_copy(nT2_sb[:],
                               ps_trs[0:EMB, 128:256]).then_inc(s_dv, 1)  # nTe
                dv.wait_ge(s_pe, PE_M[("phT", t)])
                dv.tensor_copy(phT_sb[:],
                               ps_trs[0:PRED_H, 128:256]).then_inc(s_dv, 1)  # phTe
                dv.wait_ge(s_ac, AC_M[("y", t)])
                dv.tensor_scalar(out=yall_sb[:, t:t + 1], in0=ytanh_sb[:],
                                 scalar1=0.5, scalar2=0.5, op0=MULT,
                                 op1=ADD).then_inc(s_dv, 1)           # yd

        @block.scalar
        def _(ac):
            ac.wait_ge(s_c, NC_ALL)
            for t in range(T):
                for h, (cs, ce) in enumerate(HALVES):
                    ncol = (ce - cs) * 128
                    ac.wait_ge(s_pe, PE_M[(f"z{h}", t)])
                    ac.activation(out=zr_sb[h][:, 0:ncol],
                                  in_=ps_z[:, 0:ncol],
                                  func=RELU, bias=b1c_sb[:]).then_inc(s_ac, 1)
                ac.wait_ge(s_dv, DV_M[("lm", t)])
                ac.activation(out=e_sb[:], in_=lm_sb[:],
                              func=EXP).then_inc(s_ac, 1)             # exp
                ac.wait_ge(s_pe, PE_M[("ph", t)])
                ac.activation(out=phs_sb[:], in_=ps_sm[:, 64:72],
                              func=RELU).then_inc(s_ac, 1)            # phr
                ac.wait_ge(s_pe, PE_M[("y", t)])
                ac.activation(out=ytanh_sb[:], in_=ps_sm[:, 72:73],
                              func=TANH, scale=0.5,
                              bias=0.5 * pred_b2).then_inc(s_ac, 1)   # y

    with nc.Block() as block2:

        @block2.sync
        def _(sync):
            sync.dma_start(out=out[:], in_=yall_sb[:]).then_inc(s_out, 16)
            n_out = 16
            if DBG:
                for dst, src_sb in [
                        (d_emb, emb_sb[:, (T - 1) * MAXM:T * MAXM, :]),
                        (d_memT, memT_sb[1][:, 0:12 * 128]),
                        (d_zr, zr_sb[1][:, 0:12 * 128]),
                        (d_itemT, itemT_sb[:]), (d_lm, lm_sb[:]),
                        (d_e, e_sb[:]), (d_graw, graw_sb[:]),
                        (d_new, new_sb[:])]:
                    sync.dma_start(out=dst[:], in_=src_sb).then_inc(s_out, 16)
                    n_out += 16
            sync.wait_ge(s_out, n_out)

    return nc, ctx


def prep_inputs(member_idx, member_mask, item_inputs, user_table, item_table,
                att_w1, att_b1, att_w2, att_b2, pred_w1, pred_b1, pred_w2,
                pred_b2):
    """Host-side shard + layout prep. Returns (in_maps, pred_b2_scalar)."""
    bf = ml_dtypes.bfloat16
    ut = np.ascontiguousarray(np.asarray(user_table, np.float32)).astype(bf)
    it = np.ascontiguousarray(np.asarray(item_table, np.float32)).astype(bf)
    midx = np.asarray(member_idx).astype(np.int32).clip(0, NUM_USERS - 1)
    iidx = np.asarray(item_inputs).astype(np.int32).clip(0, NUM_ITEMS - 1)
    mask = np.asarray(member_mask).astype(bool)

    att_w1 = np.asarray(att_w1, np.float32)
    w1u = att_w1[:EMB]
    w1i = att_w1[EMB:]
    att_b1 = np.asarray(att_b1, np.float32)
    att_w2v = np.asarray(att_w2, np.float32)[:, 0]
    att_b2v = float(np.asarray(att_b2, np.float32).reshape(-1)[0])
    pred_w1 = np.asarray(pred_w1, np.float32)
    pred_b1 = np.asarray(pred_b1, np.float32)
    pred_w2 = np.asarray(pred_w2, np.float32)
    pred_b2v = float(np.asarray(pred_b2, np.float32).reshape(-1)[0])

    w1u2 = np.zeros((128, 2 * ATT_H), np.float32)
    w1u2[0:EMB, 0:ATT_H] = w1u
    w1u2[EMB:128, ATT_H:2 * ATT_H] = w1u
    w1i2 = np.concatenate([w1i, w1i], axis=1)
    b1c = np.concatenate([att_b1, att_b1])[:, None]
    w2b = np.zeros((2 * ATT_H, 2), np.float32)
    w2b[0:ATT_H, 0] = att_w2v
    w2b[ATT_H:, 1] = att_w2v

    in_maps = []
    for c in range(NCORES):
        lo = c * BL
        mi = midx[lo:lo + BL]
        ii = iidx[lo:lo + BL]
        mk = mask[lo:lo + BL]
        mi_r = mi.reshape(T, TG, MAXM).transpose(1, 0, 2).reshape(128, T * MAXM)
        ii_r = ii.reshape(T, TG).transpose(1, 0)
        mk_r = mk.reshape(T, TG, MAXM).transpose(1, 0, 2)
        madd = np.where(mk_r, 0.0, NEG).astype(np.float32) + att_b2v
        in_maps.append({
            "ut": ut, "it": it,
            "midx": np.ascontiguousarray(mi_r),
            "iidx": np.ascontiguousarray(ii_r),
            "madd": np.ascontiguousarray(madd),
            "w1u2": w1u2.astype(bf), "w1i2": w1i2.astype(bf),
            "b1c": b1c.astype(np.float32), "w2b": w2b.astype(bf),
            "pw1a": pred_w1[0:128].astype(bf),
            "pw1b": pred_w1[128:192].astype(bf),
            "pb1r": pred_b1[None, :].astype(bf),
            "ones1": np.ones((1, 128), bf),
            "pw2": pred_w2.astype(bf),
            "ident": np.eye(128, dtype=np.float32).astype(bf),
        })
    return in_maps, pred_b2v


_NC_CACHE = {}


def _ensure_ntff_hook():
    """Register the axon NTFF profile hook if the image's antenv lacks it."""
    import sys
    import types
    try:
        from antenv.axon_hooks import get_axon_ntff_profile_hook  # noqa: F401
        return True
    except ImportError:
        pass
    try:
        import antenv
        from trn_agent_boot.trn_boot import _ntff_profile_via_ctypes
        hook = _ntff_profile_via_ctypes("/opt/axon/libaxon_pjrt.so")
        mod = types.ModuleType("antenv.axon_hooks")
        _h = [hook]
        mod.set_axon_ntff_profile_hook = lambda h: _h.__setitem__(0, h)
        mod.get_axon_ntff_profile_hook = lambda: _h[0]
        sys.modules["antenv.axon_hooks"] = mod
        antenv.axon_hooks = mod
        return hook is not None
    except Exception:
        return False


def _enable_vector_dge():
    """The axon-default neuronx-cc flags disable vector_dynamic_offsets
    (indirect DMA with an offset vector). Our gather needs it."""
    try:
        from concourse.compiler_utils import (get_compiler_flags,
                                              set_compiler_flags)
        flags = get_compiler_flags()
        if "vector_dynamic_offsets" not in flags:
            return
        out = []
        i = 0
        while i < len(flags):
            f = flags[i]
            if f == "--internal-disable-dge-levels":
                out.append(f)
                i += 1
                while i < len(flags) and not flags[i].startswith("-"):
                    if flags[i] != "vector_dynamic_offsets":
                        out.append(flags[i])
                    i += 1
                continue
            out.append(f)
            if f == "--internal-enable-dge-levels":
                out.append("vector_dynamic_offsets")
            i += 1
        set_compiler_flags(out)
    except Exception:
        pass


def kernel(**inputs) -> np.ndarray:
    _enable_vector_dge()
    in_maps, pred_b2 = prep_inputs(**inputs)
    if pred_b2 not in _NC_CACHE:
        _NC_CACHE[pred_b2] = build_nc(pred_b2)
    nc, _ctx = _NC_CACHE[pred_b2]
    trace = bool(int(os.environ.get("KERNEL_TRACE", "0")))
    if trace:
        trace = _ensure_ntff_hook()
    res = run_bass_kernel_spmd(nc, in_maps, core_ids=list(range(NCORES)),
                               trace=trace)
    if trace and res.exec_time_ns is not None:
        print(f"HW exec time: {res.exec_time_ns} ns")
    outs = []
    for c in range(NCORES):
        y = np.asarray(res.results[c]["out"], np.float32)
        outs.append(y.transpose(1, 0).reshape(BL, 1))
    return np.concatenate(outs, axis=0)


# revision 16
# speedup vs baseline: 1.0012x; 1.0012x over previous
"""AGREE group-recommendation kernel for 8 TRN2 NeuronCores.

Data-parallel: 8192 groups sharded 1024/core. Per core:
  - indirect-DMA gather of member embeddings (bf16, b-layout:
    partition = group-within-tile, 8 tiles x 128 groups x 50 members)
  - attention MLP via PE transposes + block-diag matmuls
  - masked softmax (no max-subtraction: logits are tiny by construction)
  - weighted member sum on DVE, prediction MLP on PE
Host side only reshapes/casts inputs and concatenates outputs.
"""

import contextlib
import os

import numpy as np
import ml_dtypes

from concourse import bass, mybir
from concourse.bass import IndirectOffsetOnAxis
from concourse.bass_utils import run_bass_kernel_spmd

F32 = mybir.dt.float32
BF16 = mybir.dt.bfloat16
I32 = mybir.dt.int32

NUM_USERS = 100000
NUM_ITEMS = 50000
EMB = 64
B = 8192
MAXM = 50
ATT_H = 16
PRED_H = 8
NCORES = 8
BL = B // NCORES          # 1024 groups per core
T = 8                     # tiles per core
TG = 128                  # groups per tile
HALVES = ((0, 13), (13, 25))  # 2-member-chunk ranges per half-tile
NEG = -30000.0            # additive mask for invalid members
RELU = mybir.ActivationFunctionType.Relu
EXP = mybir.ActivationFunctionType.Exp
TANH = mybir.ActivationFunctionType.Tanh
MULT = mybir.AluOpType.mult
ADD = mybir.AluOpType.add

# --- precomputed semaphore schedules (must match emission order below) ---
PE_ORDER = ["itemT", "T0", "z0", "lg0", "T1", "z1", "lg1", "nT", "ph", "phT", "y"]
DV_ORDER = ["itemTe", "memT0", "memT1", "lm", "new", "nTe", "phTe", "yd"]
AC_ORDER = ["zr0", "zr1", "exp", "phr", "y"]


def _marks(order):
    m = {}
    v = 0
    for t in range(T):
        for k in order:
            v += 1
            m[(k, t)] = v
    return m


PE_M = _marks(PE_ORDER)
DV_M = _marks(DV_ORDER)
AC_M = _marks(AC_ORDER)


def build_nc(pred_b2: float):
    nc = bass.Bass()

    ut = nc.declare_dram_parameter("ut", [NUM_USERS, EMB], BF16, False)
    it = nc.declare_dram_parameter("it", [NUM_ITEMS, EMB], BF16, False)
    midx = nc.declare_dram_parameter("midx", [128, T * MAXM], I32, False)
    iidx = nc.declare_dram_parameter("iidx", [128, T], I32, False)
    madd = nc.declare_dram_parameter("madd", [128, T, MAXM], F32, False)
    w1u2 = nc.declare_dram_parameter("w1u2", [128, 2 * ATT_H], BF16, False)
    w1i2 = nc.declare_dram_parameter("w1i2", [EMB, 2 * ATT_H], BF16, False)
    b1c = nc.declare_dram_parameter("b1c", [2 * ATT_H, 1], F32, False)
    w2b = nc.declare_dram_parameter("w2b", [2 * ATT_H, 2], BF16, False)
    pw1a = nc.declare_dram_parameter("pw1a", [128, PRED_H], BF16, False)
    pw1b = nc.declare_dram_parameter("pw1b", [EMB, PRED_H], BF16, False)
    pb1r = nc.declare_dram_parameter("pb1r", [1, PRED_H], BF16, False)
    ones1 = nc.declare_dram_parameter("ones1", [1, 128], BF16, False)
    pw2 = nc.declare_dram_parameter("pw2", [PRED_H, 1], BF16, False)
    ident = nc.declare_dram_parameter("ident", [128, 128], BF16, False)
    out = nc.declare_dram_parameter("out", [128, T], F32, True)
    DBG = bool(int(os.environ.get("KERNEL_DEBUG", "0")))
    if DBG:
        d_emb = nc.declare_dram_parameter("d_emb", [128, MAXM, EMB], BF16, True)
        d_memT = nc.declare_dram_parameter("d_memT", [128, 12 * 128], BF16, True)
        d_zr = nc.declare_dram_parameter("d_zr", [2 * ATT_H, 12 * 128], BF16, True)
        d_itemT = nc.declare_dram_parameter("d_itemT", [EMB, 128], BF16, True)
        d_lm = nc.declare_dram_parameter("d_lm", [128, MAXM], F32, True)
        d_e = nc.declare_dram_parameter("d_e", [128, MAXM], F32, True)
        d_graw = nc.declare_dram_parameter("d_graw", [128, EMB], F32, True)
        d_new = nc.declare_dram_parameter("d_new", [128, 3 * EMB], BF16, True)

    ctx = contextlib.ExitStack()
    sb = ctx.enter_context
    emb_sb = sb(nc.sbuf_tensor("emb_sb", [128, T * MAXM, EMB], BF16))
    item_sb = sb(nc.sbuf_tensor("item_sb", [128, T, EMB], BF16))
    midx_sb = sb(nc.sbuf_tensor("midx_sb", [128, T * MAXM], I32))
    iidx_sb = sb(nc.sbuf_tensor("iidx_sb", [128, T], I32))
    madd_sb = sb(nc.sbuf_tensor("madd_sb", [128, T, MAXM], F32))
    w1u2_sb = sb(nc.sbuf_tensor("w1u2_sb", [128, 2 * ATT_H], BF16))
    w1i2_sb = sb(nc.sbuf_tensor("w1i2_sb", [EMB, 2 * ATT_H], BF16))
    b1c_sb = sb(nc.sbuf_tensor("b1c_sb", [2 * ATT_H, 1], F32))
    w2b_sb = sb(nc.sbuf_tensor("w2b_sb", [2 * ATT_H, 2], BF16))
    pw1a_sb = sb(nc.sbuf_tensor("pw1a_sb", [128, PRED_H], BF16))
    pw1b_sb = sb(nc.sbuf_tensor("pw1b_sb", [EMB, PRED_H], BF16))
    pb1r_sb = sb(nc.sbuf_tensor("pb1r_sb", [1, PRED_H], BF16))
    ones1_sb = sb(nc.sbuf_tensor("ones1_sb", [1, 128], BF16))
    pw2_sb = sb(nc.sbuf_tensor("pw2_sb", [PRED_H, 1], BF16))
    ident_sb = sb(nc.sbuf_tensor("ident_sb", [128, 128], BF16))

    itemT_sb = sb(nc.sbuf_tensor("itemT_sb", [EMB, 128], BF16))
    memT_sb = [sb(nc.sbuf_tensor(f"memT{h}_sb", [128, 13 * 128], BF16))
               for h in range(2)]
    zr_sb = [sb(nc.sbuf_tensor(f"zr{h}_sb", [2 * ATT_H, 13 * 128], BF16))
             for h in range(2)]
    lm_sb = sb(nc.sbuf_tensor("lm_sb", [128, MAXM], F32))
    e_sb = sb(nc.sbuf_tensor("e_sb", [128, MAXM], F32))
    ssum_sb = sb(nc.sbuf_tensor("ssum_sb", [128, 1], F32))
    rre_sb = sb(nc.sbuf_tensor("rre_sb", [128, 1], F32))
    prod_sb = sb(nc.sbuf_tensor("prod_sb", [128, MAXM, EMB], F32))
    new_sb = sb(nc.sbuf_tensor("new_sb", [128, 3 * EMB], BF16))
    nT1_sb = sb(nc.sbuf_tensor("nT1_sb", [128, 128], BF16))
    nT2_sb = sb(nc.sbuf_tensor("nT2_sb", [EMB, 128], BF16))
    phs_sb = sb(nc.sbuf_tensor("phs_sb", [128, PRED_H], BF16))
    phT_sb = sb(nc.sbuf_tensor("phT_sb", [PRED_H, 128], BF16))
    graw_sb = sb(nc.sbuf_tensor("graw_sb", [128, EMB], F32))
    ytanh_sb = sb(nc.sbuf_tensor("ytanh_sb", [128, 1], F32))
    yall_sb = sb(nc.sbuf_tensor("yall_sb", [128, T], F32))

    ps_tr = sb(nc.psum_tensor("ps_tr", [128, 13 * 128], BF16))
    ps_z = sb(nc.psum_tensor("ps_z", [2 * ATT_H, 13 * 128], F32))
    ps_trs = sb(nc.psum_tensor("ps_trs", [128, 256], BF16))
    # ps_trs carve (bf16): cols 0:128 itemT/nT1, 128:256 nT2/phT
    ps_sm = sb(nc.psum_tensor("ps_sm", [128, 80], F32))
    # ps_sm carve (f32): cols 0:64 logits, 64:72 ph, 72:73 y

    s_c = ctx.enter_context(nc.semaphore("s_c"))
    s_ci = ctx.enter_context(nc.semaphore("s_ci"))
    s_dd = ctx.enter_context(nc.semaphore("s_dd"))
    s_git = ctx.enter_context(nc.semaphore("s_git"))
    s_gt = [ctx.enter_context(nc.semaphore(f"s_g{t}")) for t in range(T)]
    s_pe = ctx.enter_context(nc.semaphore("s_pe"))
    s_dv = ctx.enter_context(nc.semaphore("s_dv"))
    s_ac = ctx.enter_context(nc.semaphore("s_ac"))
    s_out = ctx.enter_context(nc.semaphore("s_out"))

    consts = [
        (madd_sb, madd), (w1u2_sb, w1u2),
        (w1i2_sb, w1i2), (b1c_sb, b1c), (w2b_sb, w2b), (pw1a_sb, pw1a),
        (pw1b_sb, pw1b), (pb1r_sb, pb1r), (ones1_sb, ones1), (pw2_sb, pw2),
        (ident_sb, ident),
    ]
    NC_ALL = 16 * len(consts)

    with nc.Block() as block:

        @block.sync
        def _(sync):
            sync.dma_start(out=midx_sb[:], in_=midx[:]).then_inc(s_ci, 16)
            sync.dma_start(out=iidx_sb[:], in_=iidx[:]).then_inc(s_ci, 16)
            for dst, src in consts:
                sync.dma_start(out=dst[:], in_=src[:]).then_inc(s_c, 16)

        @block.gpsimd
        def _(gp):
            gp.wait_ge(s_ci, 32)  # midx + iidx loaded
            for t in range(T):
                gp.indirect_dma_start(
                    out=item_sb[:, t, :], out_offset=None, in_=it[:],
                    in_offset=IndirectOffsetOnAxis(
                        ap=iidx_sb[:, t:t + 1], axis=0),
                ).then_inc(s_git, 16)
            # walrus only supports one gathered row per partition per
            # indirect DMA -> column-wise gathers (128 rows each)
            for t in range(T):
                for s in range(MAXM):
                    gp.indirect_dma_start(
                        out=emb_sb[:, t * MAXM + s, :], out_offset=None,
                        in_=ut[:],
                        in_offset=IndirectOffsetOnAxis(
                            ap=midx_sb[:, t * MAXM + s:t * MAXM + s + 1],
                            axis=0),
                    ).then_inc(s_gt[t], 16)

        @block.tensor
        def _(pe):
            pe.wait_ge(s_c, NC_ALL)
            for t in range(T):
                if t == 0:
                    pe.wait_ge(s_git, 16 * T)
                pe.wait_ge(s_gt[t], 16 * MAXM)
                if t > 0:
                    pe.wait_ge(s_dv, DV_M[("nTe", t - 1)])
                pe.matmul(out=ps_trs[0:EMB, 0:128], lhsT=item_sb[:, t, :],
                          rhs=ident_sb[:], is_transpose=True,
                          start=True, stop=True).then_inc(s_pe, 1)  # itemT
                for h, (cs, ce) in enumerate(HALVES):
                    nch = ce - cs
                    ncol = nch * 128
                    for c in range(cs, ce):
                        i = pe.matmul(
                            out=ps_tr[:, (c - cs) * 128:(c - cs + 1) * 128],
                            lhsT=emb_sb[:, t * MAXM + 2 * c:t * MAXM + 2 * c + 2, :],
                            rhs=ident_sb[:], is_transpose=True,
                            start=True, stop=True)
                    i.then_inc(s_pe, 1)                             # T{h}
                    pe.wait_ge(s_dv, DV_M[(f"memT{h}", t)])
                    if h == 0:
                        pe.wait_ge(s_dv, DV_M[("itemTe", t)])
                    # psum-bank (512 f32) chunked z matmuls
                    for lo in range(0, ncol, 512):
                        w = min(512, ncol - lo)
                        pe.matmul(out=ps_z[:, lo:lo + w], lhsT=w1u2_sb[:],
                                  rhs=memT_sb[h][:, lo:lo + w],
                                  start=True, stop=False)
                        for p in range(lo, lo + w, 128):
                            i = pe.matmul(
                                out=ps_z[:, p:p + 128],
                                lhsT=w1i2_sb[:], rhs=itemT_sb[:],
                                start=False, stop=(p + 128 >= lo + w))
                    i.then_inc(s_pe, 1)                             # z{h}
                    pe.wait_ge(s_ac, AC_M[(f"zr{h}", t)])
                    if h == 0 and t > 0:
                        pe.wait_ge(s_dv, DV_M[("lm", t - 1)])
                    for c in range(nch):
                        m0 = 2 * (cs + c)
                        i = pe.matmul(out=ps_sm[:, m0:m0 + 2],
                                      lhsT=zr_sb[h][:, c * 128:(c + 1) * 128],
                                      rhs=w2b_sb[:], start=True, stop=True)
                    i.then_inc(s_pe, 1)                             # lg{h}
                # prediction MLP
                pe.wait_ge(s_dv, DV_M[("new", t)])
                pe.matmul(out=ps_trs[:, 0:128], lhsT=new_sb[:, 0:128],
                          rhs=ident_sb[:], is_transpose=True,
                          start=True, stop=True)
                pe.matmul(out=ps_trs[0:EMB, 128:256], lhsT=new_sb[:, 128:192],
                          rhs=ident_sb[:], is_transpose=True,
                          start=True, stop=True).then_inc(s_pe, 1)  # nT
                pe.wait_ge(s_dv, DV_M[("nTe", t)])
                pe.matmul(out=ps_sm[:, 64:72], lhsT=nT1_sb[:],
                          rhs=pw1a_sb[:], start=True, stop=False)
                pe.matmul(out=ps_sm[:, 64:72], lhsT=nT2_sb[:],
                          rhs=pw1b_sb[:], start=False, stop=False)
                pe.matmul(out=ps_sm[:, 64:72], lhsT=ones1_sb[:],
                          rhs=pb1r_sb[:], start=False,
                          stop=True).then_inc(s_pe, 1)              # ph
                pe.wait_ge(s_ac, AC_M[("phr", t)])
                if t > 0:
                    pe.wait_ge(s_ac, AC_M[("y", t - 1)])
                pe.matmul(out=ps_trs[0:PRED_H, 128:256], lhsT=phs_sb[:],
                          rhs=ident_sb[:], is_transpose=True,
                          start=True, stop=True).then_inc(s_pe, 1)  # phT
                pe.wait_ge(s_dv, DV_M[("phTe", t)])
                pe.matmul(out=ps_sm[:, 72:73], lhsT=phT_sb[:],
                          rhs=pw2_sb[:], start=True,
                          stop=True).then_inc(s_pe, 1)              # y

        @block.vector
        def _(dv):
            dd = [0]
            dv.wait_ge(s_c, NC_ALL)
            for t in range(T):
                dv.wait_ge(s_pe, PE_M[("itemT", t)])
                dv.tensor_copy(itemT_sb[:],
                               ps_trs[0:EMB, 0:128]).then_inc(s_dv, 1)  # itemTe
                for h, (cs, ce) in enumerate(HALVES):
                    ncol = (ce - cs) * 128
                    dv.wait_ge(s_pe, PE_M[(f"T{h}", t)])
                    dv.tensor_copy(memT_sb[h][:, 0:ncol],
                                   ps_tr[:, 0:ncol]).then_inc(s_dv, 1)  # memT{h}
                dv.wait_ge(s_pe, PE_M[("lg1", t)])
                dv.tensor_add(lm_sb[:], ps_sm[:, 0:MAXM],
                              madd_sb[:, t, :]).then_inc(s_dv, 1)     # lm
                dv.wait_ge(s_ac, AC_M[("exp", t)])
                dv.reduce_sum(ssum_sb[:], e_sb[:],
                              axis=mybir.AxisListType.X).then_inc(s_dd, 1)
                dd[0] += 1
                dv.tensor_tensor(
                    out=prod_sb[:], in0=emb_sb[:, t * MAXM:(t + 1) * MAXM, :],
                    in1=e_sb[:].to_broadcast([128, MAXM, EMB]),
                    op=MULT).then_inc(s_dd, 1)
                dd[0] += 1
                dv.wait_ge(s_dd, dd[0] - 1)
                dv.reciprocal(rre_sb[:], ssum_sb[:]).then_inc(s_dd, 1)
                dd[0] += 1
                dv.wait_ge(s_dd, dd[0] - 1)
                dv.tensor_reduce(
                    out=graw_sb[:],
                    in_=prod_sb[:].rearrange("p m d -> p d m"),
                    axis=mybir.AxisListType.X,
                    op=ADD).then_inc(s_dd, 1)
                dd[0] += 1
                dv.wait_ge(s_dd, dd[0])
                dv.tensor_scalar(out=new_sb[:, EMB:2 * EMB],
                                 in0=graw_sb[:], scalar1=rre_sb[:],
                                 scalar2=None, op0=MULT).then_inc(s_dd, 1)
                dd[0] += 1
                dv.wait_ge(s_dd, dd[0])
                dv.tensor_tensor(out=new_sb[:, 0:EMB],
                                 in0=new_sb[:, EMB:2 * EMB],
                                 in1=item_sb[:, t, :], op=MULT)
                dv.tensor_copy(new_sb[:, 2 * EMB:3 * EMB],
                               item_sb[:, t, :]).then_inc(s_dv, 1)    # new
                dv.wait_ge(s_pe, PE_M[("nT", t)])
                dv.tensor_copy(nT1_sb[:], ps_trs[:, 0:128])
                dv.tensor_copy(nT2_sb[:],
                               ps_trs[0:EMB, 128:256]).then_inc(s_dv, 1)  # nTe
                dv.wait_ge(s_pe, PE_M[("phT", t)])
                dv.tensor_copy(phT_sb[:],
                               ps_trs[0:PRED_H, 128:256]).then_inc(s_dv, 1)  # phTe
                dv.wait_ge(s_ac, AC_M[("y", t)])
                dv.tensor_scalar(out=yall_sb[:, t:t + 1], in0=ytanh_sb[:],
                                 scalar1=0.5, scalar2=0.5, op0=MULT,
                                 op1=ADD).then_inc(s_dv, 1)           # yd

        @block.scalar
        def _(ac):
            ac.wait_ge(s_c, NC_ALL)
            for t in range(T):
                for h, (cs, ce) in enumerate(HALVES):
                    ncol = (ce - cs) * 128
                    ac.wait_ge(s_pe, PE_M[(f"z{h}", t)])
                    ac.activation(out=zr_sb[h][:, 0:ncol],
                                  in_=ps_z[:, 0:ncol],
                                  func=RELU, bias=b1c_sb[:]).then_inc(s_ac, 1)
                ac.wait_ge(s_dv, DV_M[("lm", t)])
                ac.activation(out=e_sb[:], in_=lm_sb[:],
                              func=EXP).then_inc(s_ac, 1)             # exp
                ac.wait_ge(s_pe, PE_M[("ph", t)])
                ac.activation(out=phs_sb[:], in_=ps_sm[:, 64:72],
                              func=RELU).then_inc(s_ac, 1)            # phr
                ac.wait_ge(s_pe, PE_M[("y", t)])
                ac.activation(out=ytanh_sb[:], in_=ps_sm[:, 72:73],
                              func=TANH, scale=0.5,
                              bias=0.5 * pred_b2).then_inc(s_ac, 1)   # y

    with nc.Block() as block2:

        @block2.sync
        def _(sync):
            sync.dma_start(out=out[:], in_=yall_sb[:]).then_inc(s_out, 16)
            n_out = 16
            if DBG:
                for dst, src_sb in [
                        (d_emb, emb_sb[:, (T - 1) * MAXM:T * MAXM, :]),
                        (d_memT, memT_sb[1][:, 0:12 * 128]),
                        (d_zr, zr_sb[1][:, 0:12 * 128]),
                        (d_itemT, itemT_sb[:]), (d_lm, lm_sb[:]),
                        (d_e, e_sb[:]), (d_graw, graw_sb[:]),
                        (d_new, new_sb[:])]:
                    sync.dma_start(out=dst[:], in_=src_sb).then_inc(s_out, 16)
                    n_out += 16
            sync.wait_ge(s_out, n_out)

    return nc, ctx


def prep_inputs(member_idx, member_mask, item_inputs, user_table, item_table,
                att_w1, att_b1, att_w2, att_b2, pred_w1, pred_b1, pred_w2,
                pred_b2):
    """Host-side shard + layout prep. Returns (in_maps, pred_b2_scalar)."""
    bf = ml_dtypes.bfloat16
    ut = np.ascontiguousarray(np.asarray(user_table, np.float32)).astype(bf)
    it = np.ascontiguousarray(np.asarray(item_table, np.float32)).astype(bf)
    midx = np.asarray(member_idx).astype(np.int32).clip(0, NUM_USERS - 1)
    iidx = np.asarray(item_inputs).astype(np.int32).clip(0, NUM_ITEMS - 1)
    mask = np.asarray(member_mask).astype(bool)

    att_w1 = np.asarray(att_w1, np.float32)
    w1u = att_w1[:EMB]
    w1i = att_w1[EMB:]
    att_b1 = np.asarray(att_b1, np.float32)
    att_w2v = np.asarray(att_w2, np.float32)[:, 0]
    att_b2v = float(np.asarray(att_b2, np.float32).reshape(-1)[0])
    pred_w1 = np.asarray(pred_w1, np.float32)
    pred_b1 = np.asarray(pred_b1, np.float32)
    pred_w2 = np.asarray(pred_w2, np.float32)
    pred_b2v = float(np.asarray(pred_b2, np.float32).reshape(-1)[0])

    w1u2 = np.zeros((128, 2 * ATT_H), np.float32)
    w1u2[0:EMB, 0:ATT_H] = w1u
    w1u2[EMB:128, ATT_H:2 * ATT_H] = w1u
    w1i2 = np.concatenate([w1i, w1i], axis=1)
    b1c = np.concatenate([att_b1, att_b1])[:, None]
    w2b = np.zeros((2 * ATT_H, 2), np.float32)
    w2b[0:ATT_H, 0] = att_w2v
    w2b[ATT_H:, 1] = att_w2v

    in_maps = []
    for c in range(NCORES):
        lo = c * BL
        mi = midx[lo:lo + BL]
        ii = iidx[lo:lo + BL]
        mk = mask[lo:lo + BL]
        mi_r = mi.reshape(T, TG, MAXM).transpose(1, 0, 2).reshape(128, T * MAXM)
        ii_r = ii.reshape(T, TG).transpose(1, 0)
        mk_r = mk.reshape(T, TG, MAXM).transpose(1, 0, 2)
        madd = np.where(mk_r, 0.0, NEG).astype(np.float32) + att_b2v
        in_maps.append({
            "ut": ut, "it": it,
            "midx": np.ascontiguousarray(mi_r),
            "iidx": np.ascontiguousarray(ii_r),
            "madd": np.ascontiguousarray(madd),
            "w1u2": w1u2.astype(bf), "w1i2": w1i2.astype(bf),
            "b1c": b1c.astype(np.float32), "w2b": w2b.astype(bf),
            "pw1a": pred_w1[0:128].astype(bf),
            "pw1b": pred_w1[128:192].astype(bf),
            "pb1r": pred_b1[None, :].astype(bf),
            "ones1": np.ones((1, 128), bf),
            "pw2": pred_w2.astype(bf),
            "ident": np.eye(128, dtype=np.float32).astype(bf),
        })
    return in_maps, pred_b2v


_NC_CACHE = {}


def _ensure_ntff_hook():
    """Register the axon NTFF profile hook if the image's antenv lacks it."""
    import sys
    import types
    try:
        from antenv.axon_hooks import get_axon_ntff_profile_hook  # noqa: F401
        return True
    except ImportError:
        pass
    try:
        import antenv
        from trn_agent_boot.trn_boot import _ntff_profile_via_ctypes
        hook = _ntff_profile_via_ctypes("/opt/axon/libaxon_pjrt.so")
        mod = types.ModuleType("antenv.axon_hooks")
        _h = [hook]
        mod.set_axon_ntff_profile_hook = lambda h: _h.__setitem__(0, h)
        mod.get_axon_ntff_profile_hook = lambda: _h[0]
        sys.modules["antenv.axon_hooks"] = mod
        antenv.axon_hooks = mod
        return hook is not None
    except Exception:
        return False


def _enable_vector_dge():
    """The axon-default neuronx-cc flags disable vector_dynamic_offsets
    (indirect DMA with an offset vector). Our gather needs it."""
    try:
        from concourse.compiler_utils import (get_compiler_flags,
                                              set_compiler_flags)
        flags = get_compiler_flags()
        if "vector_dynamic_offsets" not in flags:
            return
        out = []
        i = 0
        while i < len(flags):
            f = flags[i]
            if f == "--internal-disable-dge-levels":
                out.append(f)
                i += 1
                while i < len(flags) and not flags[i].startswith("-"):
                    if flags[i] != "vector_dynamic_offsets":
                        out.append(flags[i])
                    i += 1
                continue
            out.append(f)
            if f == "--internal-enable-dge-levels":
                out.append("vector_dynamic_offsets")
            i += 1
        set_compiler_flags(out)
    except Exception:
        pass


def kernel(**inputs) -> np.ndarray:
    _enable_vector_dge()
    in_maps, pred_b2 = prep_inputs(**inputs)
    if pred_b2 not in _NC_CACHE:
        _NC_CACHE[pred_b2] = build_nc(pred_b2)
    nc, _ctx = _NC_CACHE[pred_b2]
    trace = bool(int(os.environ.get("KERNEL_TRACE", "0")))
    if trace:
        trace = _ensure_ntff_hook()
    res = run_bass_kernel_spmd(nc, in_maps, core_ids=list(range(NCORES)),
                               trace=trace)
    if trace and res.exec_time_ns is not None:
        print(f"HW exec time: {res.exec_time_ns} ns")
    outs = []
    for c in range(NCORES):
        y = np.asarray(res.results[c]["out"], np.float32)
        outs.append(y.transpose(1, 0).reshape(BL, 1))
    return np.concatenate(outs, axis=0)


# revision 17
# speedup vs baseline: 1.5718x; 1.5699x over previous
"""AGREE group-recommendation kernel for 8 TRN2 NeuronCores.

Data-parallel: 8192 groups sharded 1024/core. Per core:
  - indirect-DMA gather of member embeddings (bf16, b-layout:
    partition = group-within-tile, 8 tiles x 128 groups x 50 members)
  - attention MLP via PE transposes + block-diag matmuls
  - masked softmax (no max-subtraction: logits are tiny by construction)
  - weighted member sum on DVE, prediction MLP on PE
Host side only reshapes/casts inputs and concatenates outputs.
"""

import contextlib
import os

import numpy as np
import ml_dtypes

from concourse import bass, mybir
from concourse.bass import IndirectOffsetOnAxis
from concourse.bass_utils import run_bass_kernel_spmd

F32 = mybir.dt.float32
BF16 = mybir.dt.bfloat16
I32 = mybir.dt.int32

NUM_USERS = 100000
NUM_ITEMS = 50000
EMB = 64
B = 8192
MAXM = 50
ATT_H = 16
PRED_H = 8
NCORES = 8
BL = B // NCORES          # 1024 groups per core
T = 8                     # tiles per core
TG = 128                  # groups per tile
HALVES = ((0, 13), (13, 25))  # 2-member-chunk ranges per half-tile
NEG = -30000.0            # additive mask for invalid members
RELU = mybir.ActivationFunctionType.Relu
EXP = mybir.ActivationFunctionType.Exp
TANH = mybir.ActivationFunctionType.Tanh
MULT = mybir.AluOpType.mult
ADD = mybir.AluOpType.add

# --- precomputed semaphore schedules (must match emission order below) ---
PE_ORDER = ["itemT", "T0", "z0", "lg0", "T1", "z1", "lg1", "nT", "ph", "phT", "y"]
DV_ORDER = ["itemTe", "memT0", "memT1", "lm", "new", "nTe", "phTe", "yd"]
AC_ORDER = ["zr0", "zr1", "exp", "phr", "y"]


def _marks(order):
    m = {}
    v = 0
    for t in range(T):
        for k in order:
            v += 1
            m[(k, t)] = v
    return m


PE_M = _marks(PE_ORDER)
DV_M = _marks(DV_ORDER)
AC_M = _marks(AC_ORDER)


def build_nc(pred_b2: float, ncols=(MAXM,) * T):
    nc = bass.Bass()

    ut = nc.declare_dram_parameter("ut", [NUM_USERS, EMB], BF16, False)
    it = nc.declare_dram_parameter("it", [NUM_ITEMS, EMB], BF16, False)
    midx = nc.declare_dram_parameter("midx", [128, T * MAXM], I32, False)
    iidx = nc.declare_dram_parameter("iidx", [128, T], I32, False)
    madd = nc.declare_dram_parameter("madd", [128, T, MAXM], F32, False)
    w1u2 = nc.declare_dram_parameter("w1u2", [128, 2 * ATT_H], BF16, False)
    w1i2 = nc.declare_dram_parameter("w1i2", [EMB, 2 * ATT_H], BF16, False)
    b1c = nc.declare_dram_parameter("b1c", [2 * ATT_H, 1], F32, False)
    w2b = nc.declare_dram_parameter("w2b", [2 * ATT_H, 2], BF16, False)
    pw1a = nc.declare_dram_parameter("pw1a", [128, PRED_H], BF16, False)
    pw1b = nc.declare_dram_parameter("pw1b", [EMB, PRED_H], BF16, False)
    pb1r = nc.declare_dram_parameter("pb1r", [1, PRED_H], BF16, False)
    ones1 = nc.declare_dram_parameter("ones1", [1, 128], BF16, False)
    pw2 = nc.declare_dram_parameter("pw2", [PRED_H, 1], BF16, False)
    ident = nc.declare_dram_parameter("ident", [128, 128], BF16, False)
    out = nc.declare_dram_parameter("out", [128, T], F32, True)
    DBG = bool(int(os.environ.get("KERNEL_DEBUG", "0")))
    if DBG:
        d_emb = nc.declare_dram_parameter("d_emb", [128, MAXM, EMB], BF16, True)
        d_memT = nc.declare_dram_parameter("d_memT", [128, 12 * 128], BF16, True)
        d_zr = nc.declare_dram_parameter("d_zr", [2 * ATT_H, 12 * 128], BF16, True)
        d_itemT = nc.declare_dram_parameter("d_itemT", [EMB, 128], BF16, True)
        d_lm = nc.declare_dram_parameter("d_lm", [128, MAXM], F32, True)
        d_e = nc.declare_dram_parameter("d_e", [128, MAXM], F32, True)
        d_graw = nc.declare_dram_parameter("d_graw", [128, EMB], F32, True)
        d_new = nc.declare_dram_parameter("d_new", [128, 3 * EMB], BF16, True)

    ctx = contextlib.ExitStack()
    sb = ctx.enter_context
    emb_sb = sb(nc.sbuf_tensor("emb_sb", [128, T * MAXM, EMB], BF16))
    item_sb = sb(nc.sbuf_tensor("item_sb", [128, T, EMB], BF16))
    midx_sb = sb(nc.sbuf_tensor("midx_sb", [128, T * MAXM], I32))
    iidx_sb = sb(nc.sbuf_tensor("iidx_sb", [128, T], I32))
    madd_sb = sb(nc.sbuf_tensor("madd_sb", [128, T, MAXM], F32))
    w1u2_sb = sb(nc.sbuf_tensor("w1u2_sb", [128, 2 * ATT_H], BF16))
    w1i2_sb = sb(nc.sbuf_tensor("w1i2_sb", [EMB, 2 * ATT_H], BF16))
    b1c_sb = sb(nc.sbuf_tensor("b1c_sb", [2 * ATT_H, 1], F32))
    w2b_sb = sb(nc.sbuf_tensor("w2b_sb", [2 * ATT_H, 2], BF16))
    pw1a_sb = sb(nc.sbuf_tensor("pw1a_sb", [128, PRED_H], BF16))
    pw1b_sb = sb(nc.sbuf_tensor("pw1b_sb", [EMB, PRED_H], BF16))
    pb1r_sb = sb(nc.sbuf_tensor("pb1r_sb", [1, PRED_H], BF16))
    ones1_sb = sb(nc.sbuf_tensor("ones1_sb", [1, 128], BF16))
    pw2_sb = sb(nc.sbuf_tensor("pw2_sb", [PRED_H, 1], BF16))
    ident_sb = sb(nc.sbuf_tensor("ident_sb", [128, 128], BF16))

    itemT_sb = sb(nc.sbuf_tensor("itemT_sb", [EMB, 128], BF16))
    memT_sb = [sb(nc.sbuf_tensor(f"memT{h}_sb", [128, 13 * 128], BF16))
               for h in range(2)]
    zr_sb = [sb(nc.sbuf_tensor(f"zr{h}_sb", [2 * ATT_H, 13 * 128], BF16))
             for h in range(2)]
    lm_sb = sb(nc.sbuf_tensor("lm_sb", [128, MAXM], F32))
    e_sb = sb(nc.sbuf_tensor("e_sb", [128, MAXM], F32))
    ssum_sb = sb(nc.sbuf_tensor("ssum_sb", [128, 1], F32))
    rre_sb = sb(nc.sbuf_tensor("rre_sb", [128, 1], F32))
    prod_sb = sb(nc.sbuf_tensor("prod_sb", [128, MAXM, EMB], F32))
    new_sb = sb(nc.sbuf_tensor("new_sb", [128, 3 * EMB], BF16))
    nT1_sb = sb(nc.sbuf_tensor("nT1_sb", [128, 128], BF16))
    nT2_sb = sb(nc.sbuf_tensor("nT2_sb", [EMB, 128], BF16))
    phs_sb = sb(nc.sbuf_tensor("phs_sb", [128, PRED_H], BF16))
    phT_sb = sb(nc.sbuf_tensor("phT_sb", [PRED_H, 128], BF16))
    graw_sb = sb(nc.sbuf_tensor("graw_sb", [128, EMB], F32))
    ytanh_sb = sb(nc.sbuf_tensor("ytanh_sb", [128, 1], F32))
    yall_sb = sb(nc.sbuf_tensor("yall_sb", [128, T], F32))

    ps_tr = sb(nc.psum_tensor("ps_tr", [128, 13 * 128], BF16))
    ps_z = sb(nc.psum_tensor("ps_z", [2 * ATT_H, 13 * 128], F32))
    ps_trs = sb(nc.psum_tensor("ps_trs", [128, 256], BF16))
    # ps_trs carve (bf16): cols 0:128 itemT/nT1, 128:256 nT2/phT
    ps_sm = sb(nc.psum_tensor("ps_sm", [128, 80], F32))
    # ps_sm carve (f32): cols 0:64 logits, 64:72 ph, 72:73 y

    s_c = ctx.enter_context(nc.semaphore("s_c"))
    s_ci = ctx.enter_context(nc.semaphore("s_ci"))
    s_dd = ctx.enter_context(nc.semaphore("s_dd"))
    s_git = ctx.enter_context(nc.semaphore("s_git"))
    s_gt = [ctx.enter_context(nc.semaphore(f"s_g{t}")) for t in range(T)]
    s_pe = ctx.enter_context(nc.semaphore("s_pe"))
    s_dv = ctx.enter_context(nc.semaphore("s_dv"))
    s_ac = ctx.enter_context(nc.semaphore("s_ac"))
    s_out = ctx.enter_context(nc.semaphore("s_out"))

    consts = [
        (madd_sb, madd), (w1u2_sb, w1u2),
        (w1i2_sb, w1i2), (b1c_sb, b1c), (w2b_sb, w2b), (pw1a_sb, pw1a),
        (pw1b_sb, pw1b), (pb1r_sb, pb1r), (ones1_sb, ones1), (pw2_sb, pw2),
        (ident_sb, ident),
    ]
    NC_ALL = 16 * len(consts)

    with nc.Block() as block:

        @block.sync
        def _(sync):
            sync.dma_start(out=midx_sb[:], in_=midx[:]).then_inc(s_ci, 16)
            sync.dma_start(out=iidx_sb[:], in_=iidx[:]).then_inc(s_ci, 16)
            for dst, src in consts:
                sync.dma_start(out=dst[:], in_=src[:]).then_inc(s_c, 16)

        @block.gpsimd
        def _(gp):
            gp.wait_ge(s_ci, 32)  # midx + iidx loaded
            # groups are length-sorted on host: columns >= ncols[t] are fully
            # masked in tile t -> never gathered, zero them once instead
            nms = 0
            for t in range(T):
                if ncols[t] < MAXM:
                    gp.memset(
                        emb_sb[:, t * MAXM + ncols[t]:(t + 1) * MAXM, :],
                        0).then_inc(s_ci, 1)
                    nms += 1
            for t in range(T):
                gp.indirect_dma_start(
                    out=item_sb[:, t, :], out_offset=None, in_=it[:],
                    in_offset=IndirectOffsetOnAxis(
                        ap=iidx_sb[:, t:t + 1], axis=0),
                ).then_inc(s_git, 16)
            # walrus only supports one gathered row per partition per
            # indirect DMA -> column-wise gathers (128 rows each)
            for t in range(T):
                for s in range(ncols[t]):
                    gp.indirect_dma_start(
                        out=emb_sb[:, t * MAXM + s, :], out_offset=None,
                        in_=ut[:],
                        in_offset=IndirectOffsetOnAxis(
                            ap=midx_sb[:, t * MAXM + s:t * MAXM + s + 1],
                            axis=0),
                    ).then_inc(s_gt[t], 16)

        @block.tensor
        def _(pe):
            pe.wait_ge(s_c, NC_ALL)
            for t in range(T):
                if t == 0:
                    pe.wait_ge(s_git, 16 * T)
                    pe.wait_ge(s_ci, 32 + sum(
                        1 for x in ncols if x < MAXM))
                pe.wait_ge(s_gt[t], 16 * ncols[t])
                if t > 0:
                    pe.wait_ge(s_dv, DV_M[("nTe", t - 1)])
                pe.matmul(out=ps_trs[0:EMB, 0:128], lhsT=item_sb[:, t, :],
                          rhs=ident_sb[:], is_transpose=True,
                          start=True, stop=True).then_inc(s_pe, 1)  # itemT
                for h, (cs, ce) in enumerate(HALVES):
                    nch = ce - cs
                    ncol = nch * 128
                    for c in range(cs, ce):
                        i = pe.matmul(
                            out=ps_tr[:, (c - cs) * 128:(c - cs + 1) * 128],
                            lhsT=emb_sb[:, t * MAXM + 2 * c:t * MAXM + 2 * c + 2, :],
                            rhs=ident_sb[:], is_transpose=True,
                            start=True, stop=True)
                    i.then_inc(s_pe, 1)                             # T{h}
                    pe.wait_ge(s_dv, DV_M[(f"memT{h}", t)])
                    if h == 0:
                        pe.wait_ge(s_dv, DV_M[("itemTe", t)])
                    # psum-bank (512 f32) chunked z matmuls
                    for lo in range(0, ncol, 512):
                        w = min(512, ncol - lo)
                        pe.matmul(out=ps_z[:, lo:lo + w], lhsT=w1u2_sb[:],
                                  rhs=memT_sb[h][:, lo:lo + w],
                                  start=True, stop=False)
                        for p in range(lo, lo + w, 128):
                            i = pe.matmul(
                                out=ps_z[:, p:p + 128],
                                lhsT=w1i2_sb[:], rhs=itemT_sb[:],
                                start=False, stop=(p + 128 >= lo + w))
                    i.then_inc(s_pe, 1)                             # z{h}
                    pe.wait_ge(s_ac, AC_M[(f"zr{h}", t)])
                    if h == 0 and t > 0:
                        pe.wait_ge(s_dv, DV_M[("lm", t - 1)])
                    for c in range(nch):
                        m0 = 2 * (cs + c)
                        i = pe.matmul(out=ps_sm[:, m0:m0 + 2],
                                      lhsT=zr_sb[h][:, c * 128:(c + 1) * 128],
                                      rhs=w2b_sb[:], start=True, stop=True)
                    i.then_inc(s_pe, 1)                             # lg{h}
                # prediction MLP
                pe.wait_ge(s_dv, DV_M[("new", t)])
                pe.matmul(out=ps_trs[:, 0:128], lhsT=new_sb[:, 0:128],
                          rhs=ident_sb[:], is_transpose=True,
                          start=True, stop=True)
                pe.matmul(out=ps_trs[0:EMB, 128:256], lhsT=new_sb[:, 128:192],
                          rhs=ident_sb[:], is_transpose=True,
                          start=True, stop=True).then_inc(s_pe, 1)  # nT
                pe.wait_ge(s_dv, DV_M[("nTe", t)])
                pe.matmul(out=ps_sm[:, 64:72], lhsT=nT1_sb[:],
                          rhs=pw1a_sb[:], start=True, stop=False)
                pe.matmul(out=ps_sm[:, 64:72], lhsT=nT2_sb[:],
                          rhs=pw1b_sb[:], start=False, stop=False)
                pe.matmul(out=ps_sm[:, 64:72], lhsT=ones1_sb[:],
                          rhs=pb1r_sb[:], start=False,
                          stop=True).then_inc(s_pe, 1)              # ph
                pe.wait_ge(s_ac, AC_M[("phr", t)])
                if t > 0:
                    pe.wait_ge(s_ac, AC_M[("y", t - 1)])
                pe.matmul(out=ps_trs[0:PRED_H, 128:256], lhsT=phs_sb[:],
                          rhs=ident_sb[:], is_transpose=True,
                          start=True, stop=True).then_inc(s_pe, 1)  # phT
                pe.wait_ge(s_dv, DV_M[("phTe", t)])
                pe.matmul(out=ps_sm[:, 72:73], lhsT=phT_sb[:],
                          rhs=pw2_sb[:], start=True,
                          stop=True).then_inc(s_pe, 1)              # y

        @block.vector
        def _(dv):
            dd = [0]
            dv.wait_ge(s_c, NC_ALL)
            dv.wait_ge(s_ci, 32 + sum(1 for x in ncols if x < MAXM))
            for t in range(T):
                dv.wait_ge(s_pe, PE_M[("itemT", t)])
                dv.tensor_copy(itemT_sb[:],
                               ps_trs[0:EMB, 0:128]).then_inc(s_dv, 1)  # itemTe
                for h, (cs, ce) in enumerate(HALVES):
                    ncol = (ce - cs) * 128
                    dv.wait_ge(s_pe, PE_M[(f"T{h}", t)])
                    dv.tensor_copy(memT_sb[h][:, 0:ncol],
                                   ps_tr[:, 0:ncol]).then_inc(s_dv, 1)  # memT{h}
                dv.wait_ge(s_pe, PE_M[("lg1", t)])
                dv.tensor_add(lm_sb[:], ps_sm[:, 0:MAXM],
                              madd_sb[:, t, :]).then_inc(s_dv, 1)     # lm
                dv.wait_ge(s_ac, AC_M[("exp", t)])
                dv.reduce_sum(ssum_sb[:], e_sb[:],
                              axis=mybir.AxisListType.X).then_inc(s_dd, 1)
                dd[0] += 1
                dv.tensor_tensor(
                    out=prod_sb[:], in0=emb_sb[:, t * MAXM:(t + 1) * MAXM, :],
                    in1=e_sb[:].to_broadcast([128, MAXM, EMB]),
                    op=MULT).then_inc(s_dd, 1)
                dd[0] += 1
                dv.wait_ge(s_dd, dd[0] - 1)
                dv.reciprocal(rre_sb[:], ssum_sb[:]).then_inc(s_dd, 1)
                dd[0] += 1
                dv.wait_ge(s_dd, dd[0] - 1)
                dv.tensor_reduce(
                    out=graw_sb[:],
                    in_=prod_sb[:].rearrange("p m d -> p d m"),
                    axis=mybir.AxisListType.X,
                    op=ADD).then_inc(s_dd, 1)
                dd[0] += 1
                dv.wait_ge(s_dd, dd[0])
                dv.tensor_scalar(out=new_sb[:, EMB:2 * EMB],
                                 in0=graw_sb[:], scalar1=rre_sb[:],
                                 scalar2=None, op0=MULT).then_inc(s_dd, 1)
                dd[0] += 1
                dv.wait_ge(s_dd, dd[0])
                dv.tensor_tensor(out=new_sb[:, 0:EMB],
                                 in0=new_sb[:, EMB:2 * EMB],
                                 in1=item_sb[:, t, :], op=MULT)
                dv.tensor_copy(new_sb[:, 2 * EMB:3 * EMB],
                               item_sb[:, t, :]).then_inc(s_dv, 1)    # new
                dv.wait_ge(s_pe, PE_M[("nT", t)])
                dv.tensor_copy(nT1_sb[:], ps_trs[:, 0:128])
                dv.tensor_copy(nT2_sb[:],
                               ps_trs[0:EMB, 128:256]).then_inc(s_dv, 1)  # nTe
                dv.wait_ge(s_pe, PE_M[("phT", t)])
                dv.tensor_copy(phT_sb[:],
                               ps_trs[0:PRED_H, 128:256]).then_inc(s_dv, 1)  # phTe
                dv.wait_ge(s_ac, AC_M[("y", t)])
                dv.tensor_scalar(out=yall_sb[:, t:t + 1], in0=ytanh_sb[:],
                                 scalar1=0.5, scalar2=0.5, op0=MULT,
                                 op1=ADD).then_inc(s_dv, 1)           # yd

        @block.scalar
        def _(ac):
            ac.wait_ge(s_c, NC_ALL)
            for t in range(T):
                for h, (cs, ce) in enumerate(HALVES):
                    ncol = (ce - cs) * 128
                    ac.wait_ge(s_pe, PE_M[(f"z{h}", t)])
                    ac.activation(out=zr_sb[h][:, 0:ncol],
                                  in_=ps_z[:, 0:ncol],
                                  func=RELU, bias=b1c_sb[:]).then_inc(s_ac, 1)
                ac.wait_ge(s_dv, DV_M[("lm", t)])
                ac.activation(out=e_sb[:], in_=lm_sb[:],
                              func=EXP).then_inc(s_ac, 1)             # exp
                ac.wait_ge(s_pe, PE_M[("ph", t)])
                ac.activation(out=phs_sb[:], in_=ps_sm[:, 64:72],
                              func=RELU).then_inc(s_ac, 1)            # phr
                ac.wait_ge(s_pe, PE_M[("y", t)])
                ac.activation(out=ytanh_sb[:], in_=ps_sm[:, 72:73],
                              func=TANH, scale=0.5,
                              bias=0.5 * pred_b2).then_inc(s_ac, 1)   # y

    with nc.Block() as block2:

        @block2.sync
        def _(sync):
            sync.dma_start(out=out[:], in_=yall_sb[:]).then_inc(s_out, 16)
            n_out = 16
            if DBG:
                for dst, src_sb in [
                        (d_emb, emb_sb[:, (T - 1) * MAXM:T * MAXM, :]),
                        (d_memT, memT_sb[1][:, 0:12 * 128]),
                        (d_zr, zr_sb[1][:, 0:12 * 128]),
                        (d_itemT, itemT_sb[:]), (d_lm, lm_sb[:]),
                        (d_e, e_sb[:]), (d_graw, graw_sb[:]),
                        (d_new, new_sb[:])]:
                    sync.dma_start(out=dst[:], in_=src_sb).then_inc(s_out, 16)
                    n_out += 16
            sync.wait_ge(s_out, n_out)

    return nc, ctx


def prep_inputs(member_idx, member_mask, item_inputs, user_table, item_table,
                att_w1, att_b1, att_w2, att_b2, pred_w1, pred_b1, pred_w2,
                pred_b2):
    """Host-side shard + layout prep. Returns (in_maps, pred_b2_scalar)."""
    bf = ml_dtypes.bfloat16
    ut = np.ascontiguousarray(np.asarray(user_table, np.float32)).astype(bf)
    it = np.ascontiguousarray(np.asarray(item_table, np.float32)).astype(bf)
    midx = np.asarray(member_idx).astype(np.int32).clip(0, NUM_USERS - 1)
    iidx = np.asarray(item_inputs).astype(np.int32).clip(0, NUM_ITEMS - 1)
    mask = np.asarray(member_mask).astype(bool)

    att_w1 = np.asarray(att_w1, np.float32)
    w1u = att_w1[:EMB]
    w1i = att_w1[EMB:]
    att_b1 = np.asarray(att_b1, np.float32)
    att_w2v = np.asarray(att_w2, np.float32)[:, 0]
    att_b2v = float(np.asarray(att_b2, np.float32).reshape(-1)[0])
    pred_w1 = np.asarray(pred_w1, np.float32)
    pred_b1 = np.asarray(pred_b1, np.float32)
    pred_w2 = np.asarray(pred_w2, np.float32)
    pred_b2v = float(np.asarray(pred_b2, np.float32).reshape(-1)[0])

    w1u2 = np.zeros((128, 2 * ATT_H), np.float32)
    w1u2[0:EMB, 0:ATT_H] = w1u
    w1u2[EMB:128, ATT_H:2 * ATT_H] = w1u
    w1i2 = np.concatenate([w1i, w1i], axis=1)
    b1c = np.concatenate([att_b1, att_b1])[:, None]
    w2b = np.zeros((2 * ATT_H, 2), np.float32)
    w2b[0:ATT_H, 0] = att_w2v
    w2b[ATT_H:, 1] = att_w2v

    lens = mask.sum(1)
    order = np.argsort(lens, kind="stable")      # ascending group length
    ncols = tuple(int(lens[order[min((t + 1) * NCORES * TG, B) - 1]])
                  for t in range(T))
    in_maps = []
    for c in range(NCORES):
        gids = order[np.arange(BL) * NCORES + c]   # this core's groups
        mi = midx[gids]
        ii = iidx[gids]
        mk = mask[gids]
        mi_r = mi.reshape(T, TG, MAXM).transpose(1, 0, 2).reshape(128, T * MAXM)
        ii_r = ii.reshape(T, TG).transpose(1, 0)
        mk_r = mk.reshape(T, TG, MAXM).transpose(1, 0, 2)
        madd = np.where(mk_r, 0.0, NEG).astype(np.float32) + att_b2v
        in_maps.append({
            "ut": ut, "it": it,
            "midx": np.ascontiguousarray(mi_r),
            "iidx": np.ascontiguousarray(ii_r),
            "madd": np.ascontiguousarray(madd),
            "w1u2": w1u2.astype(bf), "w1i2": w1i2.astype(bf),
            "b1c": b1c.astype(np.float32), "w2b": w2b.astype(bf),
            "pw1a": pred_w1[0:128].astype(bf),
            "pw1b": pred_w1[128:192].astype(bf),
            "pb1r": pred_b1[None, :].astype(bf),
            "ones1": np.ones((1, 128), bf),
            "pw2": pred_w2.astype(bf),
            "ident": np.eye(128, dtype=np.float32).astype(bf),
        })
    return in_maps, pred_b2v, order, ncols


_NC_CACHE = {}


def _ensure_ntff_hook():
    """Register the axon NTFF profile hook if the image's antenv lacks it."""
    import sys
    import types
    try:
        from antenv.axon_hooks import get_axon_ntff_profile_hook  # noqa: F401
        return True
    except ImportError:
        pass
    try:
        import antenv
        from trn_agent_boot.trn_boot import _ntff_profile_via_ctypes
        hook = _ntff_profile_via_ctypes("/opt/axon/libaxon_pjrt.so")
        mod = types.ModuleType("antenv.axon_hooks")
        _h = [hook]
        mod.set_axon_ntff_profile_hook = lambda h: _h.__setitem__(0, h)
        mod.get_axon_ntff_profile_hook = lambda: _h[0]
        sys.modules["antenv.axon_hooks"] = mod
        antenv.axon_hooks = mod
        return hook is not None
    except Exception:
        return False


def _enable_vector_dge():
    """The axon-default neuronx-cc flags disable vector_dynamic_offsets
    (indirect DMA with an offset vector). Our gather needs it."""
    try:
        from concourse.compiler_utils import (get_compiler_flags,
                                              set_compiler_flags)
        flags = get_compiler_flags()
        if "vector_dynamic_offsets" not in flags:
            return
        out = []
        i = 0
        while i < len(flags):
            f = flags[i]
            if f == "--internal-disable-dge-levels":
                out.append(f)
                i += 1
                while i < len(flags) and not flags[i].startswith("-"):
                    if flags[i] != "vector_dynamic_offsets":
                        out.append(flags[i])
                    i += 1
                continue
            out.append(f)
            if f == "--internal-enable-dge-levels":
                out.append("vector_dynamic_offsets")
            i += 1
        set_compiler_flags(out)
    except Exception:
        pass


def kernel(**inputs) -> np.ndarray:
    _enable_vector_dge()
    in_maps, pred_b2, order, ncols = prep_inputs(**inputs)
    key = (pred_b2, ncols)
    if key not in _NC_CACHE:
        _NC_CACHE[key] = build_nc(pred_b2, ncols)
    nc, _ctx = _NC_CACHE[key]
    trace = bool(int(os.environ.get("KERNEL_TRACE", "0")))
    if trace:
        trace = _ensure_ntff_hook()
    res = run_bass_kernel_spmd(nc, in_maps, core_ids=list(range(NCORES)),
                               trace=trace)
    if trace and res.exec_time_ns is not None:
        print(f"HW exec time: {res.exec_time_ns} ns")
    yfull = np.zeros((B, 1), np.float32)
    for c in range(NCORES):
        y = np.asarray(res.results[c]["out"], np.float32)
        ys = y.transpose(1, 0).reshape(BL)          # sorted-order results
        yfull[order[np.arange(BL) * NCORES + c], 0] = ys
    return yfull
